# revision 2
# baseline (speedup 1.0000x reference)
"""Differential Transformer kernel for TRN2, 8 cores.

Sharding: 8 cores = 2 batch groups x 4 sequence shards. Core c owns batch
c//4, rows 512*(c%4) ... +512. K/V are AllGathered within each 4-core group.

Layout convention: the residual stream lives transposed (x^T: [D, rows]) so
every dense layer is `out^T = lhsT(W).T @ rhs(x^T)` with weights in natural
[in, out] layout. Scores are computed key-major ([keys, rows]) so attn@V needs
no transposes; softmax denominators fall out of an appended ones-column on V.

Matmuls run in float32r (full PE rate, ~1.5e-4 rel err); tiles are stored as
plain f32 and bitcast to f32r at matmul call sites. The attention E/V path,
Wo and Wout run in bf16.
"""

from contextlib import ExitStack
from dataclasses import dataclass

import numpy as np

import concourse.bass as bass
import concourse.mybir as mybir
import concourse.tile as tile
from concourse.masks import make_identity

F32 = mybir.dt.float32
F32R = mybir.dt.float32r
BF16 = mybir.dt.bfloat16
AF = mybir.ActivationFunctionType
ALU = mybir.AluOpType


@dataclass
class Cfg:
    R: int = 512          # rows per core
    D: int = 1536         # model dim
    H: int = 12           # heads
    HFF: int = 4096       # ffn hidden
    V: int = 32000        # vocab
    DEPTH: int = 2
    NG: int = 4           # cores per batch group
    EPS: float = 1e-6
    LAM_INIT: float = float(0.8 - 0.6 * np.exp(-0.3 * 2))

    @property
    def HD(self):
        return self.D // self.H     # must be 128

    @property
    def D2(self):
        return 2 * self.D           # qkv out dim (H * 2*HD)

    @property
    def RT(self):
        return self.R // 128

    @property
    def DT(self):
        return self.D // 128

    @property
    def FT(self):
        return self.D2 // 128

    @property
    def KEYS(self):
        return self.NG * self.R

    @property
    def KC(self):
        return self.KEYS // 128

    @property
    def HFT(self):
        return self.HFF // 128


def _vchunks(V):
    out = []
    off = 0
    while off < V:
        out.append((off, min(512, V - off)))
        off += 512
    return out


def r_(ap):
    return ap.bitcast(F32R)


def build_kernel(tc: tile.TileContext, ins: dict, outs: dict, cfg: Cfg,
                 replica_groups):
    nc = tc.nc
    c = cfg


    assert c.HD == 128
    scale = c.HD ** -0.5

    ctx = ExitStack()
    with ctx:
        resid = ctx.enter_context(tc.tile_pool(name="resid", bufs=2))
        acts = ctx.enter_context(tc.tile_pool(name="acts", bufs=1))
        qh_p = ctx.enter_context(tc.tile_pool(name="qh", bufs=1))
        eh_p = ctx.enter_context(tc.tile_pool(name="eh", bufs=3))
        kh_p = ctx.enter_context(tc.tile_pool(name="kh", bufs=4))
        vh_p = ctx.enter_context(tc.tile_pool(name="vh", bufs=2))
        osb_p = ctx.enter_context(tc.tile_pool(name="osb", bufs=1))
        oT_p = ctx.enter_context(tc.tile_pool(name="oT", bufs=1))
        scr_p = ctx.enter_context(tc.tile_pool(name="scr", bufs=3))
        st_p = ctx.enter_context(tc.tile_pool(name="st", bufs=4))
        v1_p = ctx.enter_context(tc.tile_pool(name="v1", bufs=2))
        w_p = ctx.enter_context(tc.tile_pool(name="wp", bufs=2))
        ws_p = ctx.enter_context(tc.tile_pool(name="ws", bufs=2))
        stg_p = ctx.enter_context(tc.tile_pool(name="stg", bufs=3))
        misc_p = ctx.enter_context(tc.tile_pool(name="misc", bufs=1))
        ft_p = ctx.enter_context(tc.tile_pool(name="ftp", bufs=2))
        ps = ctx.enter_context(tc.tile_pool(name="ps", bufs=1, space="PSUM"))
        psm = ctx.enter_context(tc.tile_pool(name="psm", bufs=2, space="PSUM"))
        pss = ctx.enter_context(tc.tile_pool(name="pss", bufs=2, space="PSUM"))
        dram = ctx.enter_context(tc.tile_pool(name="dram", bufs=2, space="DRAM"))

        # constants
        ident = misc_p.tile([128, 128], F32, tag="ident")
        make_identity(nc, ident)
        ones1f = misc_p.tile([1, 128], F32, tag="ones1f")
        nc.vector.memset(ones1f, 1.0)
        ones1 = misc_p.tile([1, 128], F32R, tag="ones1")
        nc.vector.tensor_copy(ones1, ones1f)
        ones128f = misc_p.tile([128, 1], F32, tag="ones128f")
        nc.vector.memset(ones128f, 1.0)
        ones128 = misc_p.tile([128, 1], F32R, tag="ones128")
        nc.vector.tensor_copy(ones128, ones128f)
        eps1 = misc_p.tile([1, 1], F32, tag="eps1")
        nc.vector.memset(eps1, c.EPS)
        eps128 = misc_p.tile([128, 1], F32, tag="eps128")
        nc.vector.memset(eps128, c.EPS)
        nlam = misc_p.tile([128, c.DEPTH * c.H], F32, tag="nlam")
        nc.sync.dma_start(out=nlam,
                          in_=ins["neglam"].to_broadcast((128, c.DEPTH * c.H)))

        # residual stream x^T as [128, DT, R]
        xT = resid.tile([128, c.DT, c.R], F32, tag="resid")
        nc.sync.dma_start(out=xT, in_=ins["xT"].rearrange("(t p) r -> p t r", p=128))

        def rms_T(src, tag="hT"):
            """src [128, DT, R] f32 -> rms-normalized h^T [128, DT, R] f32."""
            h = acts.tile([128, c.DT, c.R], F32R, tag=tag)
            ssq = psm.tile([1, c.R], F32, tag="mm")
            for t in range(c.DT):
                sqv = scr_p.tile([128, c.R], F32R, tag="rsq")
                nc.vector.tensor_mul(sqv, src[:, t, :], src[:, t, :])
                nc.tensor.matmul(ssq, ones128, sqv,
                                 start=(t == 0), stop=(t == c.DT - 1),
                                 skip_group_check=True)
            sq_sb = v1_p.tile([1, c.R], F32, tag="v1")
            nc.scalar.activation(sq_sb, ssq, AF.Sqrt, bias=eps1, scale=1.0 / c.D)
            rs = v1_p.tile([1, c.R], F32R, tag="v1")
            with nc.allow_low_precision(reason="f32r rhs for rms scale bcast"):
                nc.vector.reciprocal(rs, sq_sb)
            bc_ps = psm.tile([128, c.R], F32, tag="mm")
            nc.tensor.matmul(bc_ps, ones1, rs, start=True, stop=True)
            bc = scr_p.tile([128, c.R], F32, tag="rsq")
            nc.vector.tensor_copy(bc, bc_ps)
            for t in range(c.DT):
                nc.vector.tensor_mul(h[:, t, :], src[:, t, :], bc)
            return h

        NCH = c.D2 // 512  # 512-chunks over qkv out dim

        for layer in range(c.DEPTH):
            wq, wk, wv = ins[f"wq{layer}"], ins[f"wk{layer}"], ins[f"wv{layer}"]
            wo = ins[f"wo{layer}"]

            hT = rms_T(xT, tag="hT")

            # ---- K^T: [D2, R] staged to DRAM f32, then gathered ----
            k_dram = dram.tile([c.D2, c.R], F32, tag="k_dram")
            for m in range(c.FT):
                wkc = w_p.tile([128, c.DT, 128], F32R, tag="wbig")
                nc.sync.dma_start(
                    out=wkc,
                    in_=wk[:, m * 128:(m + 1) * 128].rearrange("(t p) f -> p t f", p=128))
                kps = psm.tile([128, c.R], F32, tag="mm")
                for t in range(c.DT):
                    nc.tensor.matmul(kps, wkc[:, t, :], hT[:, t, :],
                                     start=(t == 0), stop=(t == c.DT - 1))
                kst = stg_p.tile([128, c.R], F32R, tag="stg")
                nc.scalar.copy(kst, kps)
                nc.sync.dma_start(out=k_dram[m * 128:(m + 1) * 128, :].bitcast(F32R), in_=kst)

            # ---- V natural with ones column: [R, H*257] bf16 to DRAM ----
            v_dram = dram.tile([c.R, c.H * 257], BF16, tag="v_dram")
            for n in range(NCH):
                vps = ps.tile([128, c.RT, 512], F32, tag="oacc")
                for t in range(c.DT):
                    wvc = ws_p.tile([128, 512], F32R, tag="wsm")
                    nc.sync.dma_start(
                        out=wvc,
                        in_=wv[t * 128:(t + 1) * 128, n * 512:(n + 1) * 512])
                    for rt in range(c.RT):
                        nc.tensor.matmul(vps[:, rt, :],
                                         hT[:, t, rt * 128:(rt + 1) * 128],
                                         wvc,
                                         start=(t == 0), stop=(t == c.DT - 1),
                                         skip_group_check=True)
                for rt in range(c.RT):
                    vst = stg_p.tile([128, 2, 257], BF16, tag="stg")
                    nc.vector.memset(vst[:, :, 256:257], 1.0)
                    nc.vector.tensor_copy(
                        vst[:, :, 0:256],
                        vps[:, rt, :].rearrange("p (h e) -> p h e", e=256))
                    nc.sync.dma_start(
                        out=v_dram[rt * 128:(rt + 1) * 128,
                                   2 * n * 257:(2 * n + 2) * 257],
                        in_=vst)

            # ---- AllGather K and V within the batch group ----
            kg = dram.tile([c.NG * c.D2, c.R], F32, tag="kg")
            nc.gpsimd.collective_compute(
                "AllGather", ALU.bypass, replica_groups=replica_groups,
                ins=[k_dram.opt()], outs=[kg.opt()])
            vg = dram.tile([c.NG * c.R, c.H * 257], BF16, tag="vg")
            nc.gpsimd.collective_compute(
                "AllGather", ALU.bypass, replica_groups=replica_groups,
                ins=[v_dram.opt()], outs=[vg.opt()])

            # ---- heads ----
            oT = oT_p.tile([128, c.FT, c.R], BF16, tag="oT")
            for h in range(c.H):
                qhead = qh_p.tile([128, 2, c.R], F32R, tag="qh")
                for a in range(2):
                    wqc = w_p.tile([128, c.DT, 128], F32R, tag="wbig")
                    nc.sync.dma_start(
                        out=wqc,
                        in_=wq[:, h * 256 + a * 128:h * 256 + (a + 1) * 128]
                        .rearrange("(t p) f -> p t f", p=128))
                    qps = psm.tile([128, c.R], F32, tag="mm")
                    for t in range(c.DT):
                        nc.tensor.matmul(qps, wqc[:, t, :], hT[:, t, :],
                                         start=(t == 0), stop=(t == c.DT - 1))
                    nc.scalar.copy(qhead[:, a, :], qps)
                vhead = vh_p.tile([128, c.KC, 257], BF16, tag="vh")
                for kc in range(c.KC):
                    nc.sync.dma_start(
                        out=vhead[:, kc, :],
                        in_=vg[kc * 128:(kc + 1) * 128, h * 257:(h + 1) * 257])

                o1sb = osb_p.tile([128, c.RT, 257], F32, tag="osb")
                for a in range(2):
                    ops = ps.tile([128, c.RT, 512], F32, tag="oacc")
                    for kc in range(c.KC):
                        rank, col = divmod(kc, c.RT)
                        kh = kh_p.tile([128, 128], F32R, tag="kh")
                        nc.sync.dma_start(
                            out=kh,
                            in_=kg[rank * c.D2 + h * 256 + a * 128:
                                   rank * c.D2 + h * 256 + (a + 1) * 128,
                                   col * 128:(col + 1) * 128].bitcast(F32R))
                        sps = pss.tile([128, c.R], F32, tag="sc")
                        nc.tensor.matmul(sps, kh, qhead[:, a, :],
                                         start=True, stop=True)
                        ee = eh_p.tile([128, c.R], BF16, tag="eh")
                        nc.scalar.activation(ee, sps, AF.Exp, scale=scale)
                        for rt in range(c.RT):
                            nc.tensor.matmul(
                                ops[:, rt, 0:257],
                                ee[:, rt * 128:(rt + 1) * 128], vhead[:, kc, :],
                                start=(kc == 0), stop=(kc == c.KC - 1),
                                skip_group_check=True)
                    if a == 0:
                        for rt in range(c.RT):
                            nc.scalar.copy(o1sb[:, rt, :], ops[:, rt, 0:257])
                    else:
                        for rt in range(c.RT):
                            r1 = st_p.tile([128, 1], F32, tag="r1")
                            nc.vector.reciprocal(r1, o1sb[:, rt, 256:257])
                            r2 = st_p.tile([128, 1], F32, tag="r2")
                            nc.vector.reciprocal(r2, ops[:, rt, 256:257])
                            sc2 = st_p.tile([128, 1], F32, tag="sc2")
                            nc.vector.tensor_mul(
                                sc2, r2,
                                nlam[:, layer * c.H + h:layer * c.H + h + 1])
                            t1 = scr_p.tile([128, 256], F32, tag="s256")
                            nc.vector.tensor_scalar_mul(t1, o1sb[:, rt, 0:256], r1)
                            oc = scr_p.tile([128, 256], F32, tag="s256")
                            nc.vector.scalar_tensor_tensor(
                                out=oc, in0=ops[:, rt, 0:256], scalar=sc2, in1=t1,
                                op0=ALU.mult, op1=ALU.add)
                            ssq = st_p.tile([128, 1], F32, tag="ssq")
                            sqo = scr_p.tile([128, 256], F32, tag="s256")
                            nc.vector.tensor_mul(sqo, oc, oc)
                            nc.vector.reduce_sum(ssq, sqo,
                                                 axis=mybir.AxisListType.X)
                            sqr = st_p.tile([128, 1], F32, tag="sqr")
                            nc.scalar.activation(sqr, ssq, AF.Sqrt,
                                                 bias=eps128, scale=1.0 / 256)
                            rinv = st_p.tile([128, 1], F32, tag="rinv")
                            nc.vector.reciprocal(rinv, sqr)
                            onr = scr_p.tile([128, 256], F32, tag="s256")
                            nc.vector.tensor_scalar_mul(onr, oc, rinv)
                            for half in range(2):
                                tp = pss.tile([128, 128], F32, tag="sc")
                                nc.tensor.transpose(
                                    tp, onr[:, half * 128:(half + 1) * 128], ident)
                                nc.vector.tensor_copy(
                                    oT[:, 2 * h + half, rt * 128:(rt + 1) * 128], tp)

            # ---- Wo + residual ----
            yT = resid.tile([128, c.DT, c.R], F32, tag="resid")
            for m in range(c.DT):
                woc = w_p.tile([128, c.FT, 128], BF16, tag="wbig")
                nc.sync.dma_start(
                    out=woc,
                    in_=wo[:, m * 128:(m + 1) * 128].rearrange("(t p) f -> p t f", p=128))
                yps = psm.tile([128, c.R], F32, tag="mm")
                for ft in range(c.FT):
                    nc.tensor.matmul(yps, woc[:, ft, :], oT[:, ft, :],
                                     start=(ft == 0), stop=(ft == c.FT - 1))
                nc.vector.tensor_add(yT[:, m, :], yps, xT[:, m, :])

            # ---- FFN (shared weights across layers) ----
            h2T = rms_T(yT, tag="hT")
            xnT = resid.tile([128, c.DT, c.R], F32, tag="resid")
            NBLK = c.HFT // 2
            for blk in range(NBLK):
                ftile = ft_p.tile([128, 2, c.R], F32R, tag="ft")
                for j in range(2):
                    m = blk * 2 + j
                    w1c = w_p.tile([128, c.DT, 128], F32R, tag="wbig")
                    nc.sync.dma_start(
                        out=w1c,
                        in_=ins["w1"][:, m * 128:(m + 1) * 128]
                        .rearrange("(t p) f -> p t f", p=128))
                    w3c = ws_p.tile([128, c.DT, 128], F32R, tag="wsm")
                    nc.sync.dma_start(
                        out=w3c,
                        in_=ins["w3"][:, m * 128:(m + 1) * 128]
                        .rearrange("(t p) f -> p t f", p=128))
                    gps = psm.tile([128, c.R], F32, tag="mm")
                    for t in range(c.DT):
                        nc.tensor.matmul(gps, w1c[:, t, :], h2T[:, t, :],
                                         start=(t == 0), stop=(t == c.DT - 1))
                    ups = pss.tile([128, c.R], F32, tag="sc")
                    for t in range(c.DT):
                        nc.tensor.matmul(ups, w3c[:, t, :], h2T[:, t, :],
                                         start=(t == 0), stop=(t == c.DT - 1))
                    gsg = scr_p.tile([128, c.R], F32, tag="g512")
                    nc.scalar.activation(gsg, gps, AF.Sigmoid)
                    gsil = scr_p.tile([128, c.R], F32, tag="g512")
                    nc.vector.tensor_mul(gsil, gsg, gps)
                    nc.vector.tensor_mul(ftile[:, j, :], gsil, ups)
                for m2 in range(c.DT):
                    w2c = ws_p.tile([128, 2, 128], F32R, tag="w2c")
                    nc.sync.dma_start(
                        out=w2c,
                        in_=ins["w2"][blk * 256:(blk + 1) * 256,
                                      m2 * 128:(m2 + 1) * 128]
                        .rearrange("(t p) f -> p t f", p=128))
                    yp2 = psm.tile([128, c.R], F32, tag="mm")
                    for j in range(2):
                        nc.tensor.matmul(yp2, w2c[:, j, :], ftile[:, j, :],
                                         start=(j == 0), stop=(j == 1))
                    if blk == 0:
                        nc.vector.tensor_add(xnT[:, m2, :], yp2, yT[:, m2, :])
                    else:
                        nc.vector.tensor_add(xnT[:, m2, :], yp2, xnT[:, m2, :])
            xT = xnT

        # ---- final projection: out = x @ Wout + bout ----
        xbf = acts.tile([128, c.DT, c.R], BF16, tag="hT")
        for t in range(c.DT):
            nc.vector.tensor_copy(xbf[:, t, :], xT[:, t, :])
        out_d = outs["out"]
        DTH = c.DT // 2
        for ci, (voff, vn) in enumerate(_vchunks(c.V)):
            woucs = []
            for hf in range(2):
                wouc = w_p.tile([128, c.DT // 2, 512], BF16, tag="wbig")
                nc.sync.dma_start(
                    out=wouc[:, :, 0:vn],
                    in_=ins["wout"][hf * DTH * 128:(hf + 1) * DTH * 128,
                                    voff:voff + vn]
                    .rearrange("(t p) f -> p t f", p=128))
                woucs.append(wouc)
            bch = v1_p.tile([1, 512], F32R, tag="v1")
            nc.sync.dma_start(out=bch[:, 0:vn], in_=ins["bout"][:, voff:voff + vn])
            for rt in range(c.RT):
                op = psm.tile([128, 512], F32, tag="mm")
                for t in range(c.DT):
                    nc.tensor.matmul(op[:, 0:vn],
                                     xbf[:, t, rt * 128:(rt + 1) * 128],
                                     woucs[t // DTH][:, t % DTH, 0:vn],
                                     start=(t == 0), stop=False,
                                     skip_group_check=True)
                nc.tensor.matmul(op[:, 0:vn], ones1, bch[:, 0:vn],
                                 start=False, stop=True, skip_group_check=True)
                ost = stg_p.tile([128, 512], F32, tag="stg")
                if (ci + rt) % 2 == 0:
                    nc.vector.tensor_copy(ost[:, 0:vn], op[:, 0:vn])
                else:
                    nc.scalar.copy(ost[:, 0:vn], op[:, 0:vn])
                nc.sync.dma_start(
                    out=out_d[rt * 128:(rt + 1) * 128, voff:voff + vn],
                    in_=ost[:, 0:vn])


def host_inputs(cfg: Cfg, core: int, x, Wq, Wk, Wv, lq1, lq2, lk1, lk2, Wo,
                w1, w2, w3, Wout, bout):
    """Build the per-core input map (numpy) from full fp32 inputs."""
    import ml_dtypes
    c = cfg
    b, sh = divmod(core, c.NG)
    rows = slice(sh * c.R, (sh + 1) * c.R)
    xT = np.ascontiguousarray(x[b, rows, :].T).astype(np.float32)
    lam = (np.exp(np.sum(lq1 * lk1, -1)) + np.exp(np.sum(lq2 * lk2, -1))
           + c.LAM_INIT)  # [DEPTH, H]
    inm = {
        "xT": xT,
        "neglam": np.ascontiguousarray(-lam.reshape(1, -1)).astype(np.float32),
        "w1": np.ascontiguousarray(w1).astype(np.float32),
        "w2": np.ascontiguousarray(w2).astype(np.float32),
        "w3": np.ascontiguousarray(w3).astype(np.float32),
        "wout": np.ascontiguousarray(Wout).astype(ml_dtypes.bfloat16),
        "bout": np.ascontiguousarray(bout.reshape(1, -1)).astype(np.float32),
    }
    for l in range(c.DEPTH):
        inm[f"wq{l}"] = np.ascontiguousarray(
            Wq[l].transpose(1, 0, 2).reshape(c.D, c.D2)).astype(np.float32)
        inm[f"wk{l}"] = np.ascontiguousarray(
            Wk[l].transpose(1, 0, 2).reshape(c.D, c.D2)).astype(np.float32)
        inm[f"wv{l}"] = np.ascontiguousarray(
            Wv[l].transpose(1, 0, 2).reshape(c.D, c.D2)).astype(np.float32)
        inm[f"wo{l}"] = np.ascontiguousarray(
            Wo[l] * (1.0 - c.LAM_INIT)).astype(ml_dtypes.bfloat16)
    return inm


def input_specs(cfg: Cfg):
    c = cfg
    sp = {
        "xT": ([c.D, c.R], F32),
        "neglam": ([1, c.DEPTH * c.H], F32),
        "w1": ([c.D, c.HFF], F32R),
        "w2": ([c.HFF, c.D], F32R),
        "w3": ([c.D, c.HFF], F32R),
        "wout": ([c.D, c.V], BF16),
        "bout": ([1, c.V], F32R),
    }
    for l in range(c.DEPTH):
        sp[f"wq{l}"] = ([c.D, c.D2], F32R)
        sp[f"wk{l}"] = ([c.D, c.D2], F32R)
        sp[f"wv{l}"] = ([c.D, c.D2], F32R)
        sp[f"wo{l}"] = ([c.D2, c.D], BF16)
    return sp


def numpy_reference(cfg: Cfg, x, Wq, Wk, Wv, lq1, lq2, lk1, lk2, Wo,
                    w1, w2, w3, Wout, bout):
    """fp64 numpy replica of reference.py for arbitrary dims."""
    c = cfg
    x = x.astype(np.float64)

    def rms(v):
        return v / np.sqrt((v * v).mean(-1, keepdims=True) + c.EPS)

    def softmax(a):
        a = a - a.max(-1, keepdims=True)
        e = np.exp(a)
        return e / e.sum(-1, keepdims=True)

    for l in range(c.DEPTH):
        h = rms(x)
        q = np.einsum('bsd,hde->bhse', h, Wq[l].astype(np.float64))
        k = np.einsum('bsd,hde->bhse', h, Wk[l].astype(np.float64))
        v = np.einsum('bsd,hde->bhse', h, Wv[l].astype(np.float64))
        Q1, Q2 = q[..., :c.HD], q[..., c.HD:]
        K1, K2 = k[..., :c.HD], k[..., c.HD:]
        sc = c.HD ** -0.5
        A1 = np.einsum('bhse,bhte->bhst', Q1, K1) * sc
        A2 = np.einsum('bhse,bhte->bhst', Q2, K2) * sc
        lam = (np.exp(np.sum(lq1[l] * lk1[l], -1))
               + np.exp(np.sum(lq2[l] * lk2[l], -1)) + c.LAM_INIT)
        attn = softmax(A1) - lam[None, :, None, None] * softmax(A2)
        o = np.einsum('bhst,bhte->bhse', attn, v)
        o = rms(o)
        o = o.transpose(0, 2, 1, 3).reshape(x.shape[0], x.shape[1], c.D2)
        o = o * (1.0 - c.LAM_INIT)
        y = o @ Wo[l].astype(np.float64) + x
        h2 = rms(y)
        g = h2 @ w1.astype(np.float64)
        x = (g / (1 + np.exp(-g))) * (h2 @ w3.astype(np.float64)) @ w2.astype(np.float64) + y
    return x @ Wout.astype(np.float64) + bout.astype(np.float64)


def make_small_inputs(cfg: Cfg, seed=0):
    """Random full inputs at cfg's dims (mimics setup_inputs)."""
    c = cfg
    rng = np.random.default_rng(seed)
    std = 0.02
    B = 2
    S = c.R * c.NG
    x = rng.standard_normal((B, S, c.D)).astype(np.float32)
    Wq = (rng.standard_normal((c.DEPTH, c.H, c.D, 2 * c.HD)) * std).astype(np.float32)
    Wk = (rng.standard_normal((c.DEPTH, c.H, c.D, 2 * c.HD)) * std).astype(np.float32)
    Wv = (rng.standard_normal((c.DEPTH, c.H, c.D, 2 * c.HD)) * std).astype(np.float32)
    lq1 = (rng.standard_normal((c.DEPTH, c.H, c.HD)) * 0.1).astype(np.float32)
    lq2 = (rng.standard_normal((c.DEPTH, c.H, c.HD)) * 0.1).astype(np.float32)
    lk1 = (rng.standard_normal((c.DEPTH, c.H, c.HD)) * 0.1).astype(np.float32)
    lk2 = (rng.standard_normal((c.DEPTH, c.H, c.HD)) * 0.1).astype(np.float32)
    Wo = (rng.standard_normal((c.DEPTH, c.D2, c.D)) * std).astype(np.float32)
    w1 = (rng.standard_normal((c.D, c.HFF)) * std).astype(np.float32)
    w2 = (rng.standard_normal((c.HFF, c.D)) * std).astype(np.float32)
    w3 = (rng.standard_normal((c.D, c.HFF)) * std).astype(np.float32)
    Wout = (rng.standard_normal((c.D, c.V)) * std).astype(np.float32)
    bout = np.zeros((c.V,), np.float32)
    return dict(x=x, Wq=Wq, Wk=Wk, Wv=Wv, lq1=lq1, lq2=lq2, lk1=lk1, lk2=lk2,
                Wo=Wo, w1=w1, w2=w2, w3=w3, Wout=Wout, bout=bout)


# ======================================================================
# Harness entry point: kernel(**inputs) -> full output [2, 2048, 32000]
# ======================================================================

_BUILT = {}


def _build_nc(repeat=1):
    from concourse import bacc
    cfg = Cfg()
    rg = [[0, 1, 2, 3], [4, 5, 6, 7]]
    nc = bacc.Bacc("TRN2", target_bir_lowering=False, debug=False,
                   num_devices=8)
    ins_ap, outs_ap = {}, {}
    for name, (shape, dt) in input_specs(cfg).items():
        ins_ap[name] = nc.dram_tensor(name, shape, dt,
                                      kind="ExternalInput").ap()
    outs_ap["out"] = nc.dram_tensor("out", [cfg.R, cfg.V], mybir.dt.float32,
                                    kind="ExternalOutput").ap()
    with tile.TileContext(nc) as tc:
        for _ in range(repeat):
            build_kernel(tc, ins_ap, outs_ap, cfg, rg)
    nc.compile()
    return cfg, nc


def kernel(x, Wq, Wk, Wv, lq1, lq2, lk1, lk2, Wo, w1, w2, w3, Wout, bout):
    from concourse.bass_utils import run_bass_kernel_spmd
    if "nc" not in _BUILT:
        _BUILT["cfg"], _BUILT["nc"] = _build_nc()
    cfg, nc = _BUILT["cfg"], _BUILT["nc"]
    args = dict(x=np.asarray(x, np.float32), Wq=np.asarray(Wq, np.float32),
                Wk=np.asarray(Wk, np.float32), Wv=np.asarray(Wv, np.float32),
                lq1=np.asarray(lq1, np.float32), lq2=np.asarray(lq2, np.float32),
                lk1=np.asarray(lk1, np.float32), lk2=np.asarray(lk2, np.float32),
                Wo=np.asarray(Wo, np.float32), w1=np.asarray(w1, np.float32),
                w2=np.asarray(w2, np.float32), w3=np.asarray(w3, np.float32),
                Wout=np.asarray(Wout, np.float32),
                bout=np.asarray(bout, np.float32))
    in_maps = [host_inputs(cfg, core, **args) for core in range(8)]
    r = run_bass_kernel_spmd(nc, in_maps, core_ids=list(range(8)))
    B, S = 2, cfg.R * cfg.NG
    out = np.empty((B, S, cfg.V), np.float32)
    for core in range(8):
        b, sh = divmod(core, cfg.NG)
        out[b, sh * cfg.R:(sh + 1) * cfg.R, :] = r.results[core]["out"]
    return out

